# revision 1
# baseline (speedup 1.0000x reference)
"""GQA multi-head attention (B=2, S=2048, D=2048, HQ=16, HKV=4, DK=128) with
RoPE + causal softmax + output projection, sharded over 8 NeuronCores as
(batch x kv-head-group): core c handles batch c//4, kv head c%4 (4 query
heads). w_q/w_kv column-sharded, fc row-sharded; partial fc outputs are
summed on the host (the "all-reduce").
"""

import sys

for _p in ("/opt/trn_rl_repo", "/root/.axon_site", "/root/.axon_site/_ro/trn_rl_repo"):
    if _p not in sys.path:
        sys.path.insert(0, _p)

import numpy as np

import concourse.bass as bass
import concourse.mybir as mybir
import concourse.tile as tile
from concourse import bacc
from concourse.bass_utils import run_bass_kernel_spmd

F32 = mybir.dt.float32
F16 = mybir.dt.float16

B, S, D = 2, 2048, 2048
HKV, NREP, DK = 4, 4, 128
HG = NREP  # query heads per core
KC = D // 128  # contraction chunks
SQC = S // 512  # 512-wide query column chunks
SCALE = float(1.0 / np.sqrt(DK))

_COMPILED = None


def _build():
    nc = bacc.Bacc(None, target_bir_lowering=False, debug=False)

    xT = nc.dram_tensor("xT", [D, S], F16, kind="ExternalInput")
    wq = nc.dram_tensor("wq", [D, HG * DK], F16, kind="ExternalInput")
    wk = nc.dram_tensor("wk", [D, DK], F16, kind="ExternalInput")
    wv = nc.dram_tensor("wv", [D, DK], F16, kind="ExternalInput")
    fcw = nc.dram_tensor("fcw", [HG * DK, D], F16, kind="ExternalInput")
    cosT = nc.dram_tensor("cosT", [128, S], F16, kind="ExternalInput")
    sinT = nc.dram_tensor("sinT", [128, S], F16, kind="ExternalInput")
    masks = nc.dram_tensor("masks", [128, 4, 512], F16, kind="ExternalInput")
    onesc = nc.dram_tensor("onesc", [128, 1], F16, kind="ExternalInput")
    iden = nc.dram_tensor("iden", [128, 128], F16, kind="ExternalInput")
    out = nc.dram_tensor("out", [S, D], F32, kind="ExternalOutput")

    with tile.TileContext(nc) as tc:
        with tc.tile_pool(name="persist", bufs=1) as persist:
            # attention-phase residents
            qt_sb = persist.tile([128, HG, S], F16)  # Q^T, rope'd, per head
            kt_sb = persist.tile([128, S], F16)  # K^T rope'd
            v_sb = persist.tile([128, KC, DK], F16)  # V  [sk, dk] chunks
            ctxT = persist.tile([128, HG, S], F16)  # (softmax @ V)^T per head
            cos_sb = persist.tile([128, S], F16)
            sin_sb = persist.tile([128, S], F16)
            mask_sb = persist.tile([128, 4, 512], F16)
            ones_sb = persist.tile([128, 1], F16)
            iden_sb = persist.tile([128, 128], F16)
            fcw_sb = persist.tile([128, HG, D], F16)

            # pools shared across all phases (no release/realloc barriers)
            ps8 = tc.alloc_tile_pool(name="ps8", bufs=8, space="PSUM")
            es_pool = tc.alloc_tile_pool(name="es_pool", bufs=5)
            nrm_pool = tc.alloc_tile_pool(name="nrm_pool", bufs=3)

            with tc.tile_pool(name="p1sb", bufs=1) as p1sb, \
                 tc.tile_pool(name="p1tmp", bufs=2) as p1tmp:
                xt_sb = p1sb.tile([128, KC, S], F16)
                wq_sb = p1sb.tile([128, KC, HG * DK], F16)
                wk_sb = p1sb.tile([128, KC, DK], F16)
                wv_sb = p1sb.tile([128, KC, DK], F16)
                vt_sb = p1sb.tile([128, S], F16)

                # DMA priority order. ACT ring: weights in consumption order;
                # SP ring: the 16 xT chunks. Q-projection matmuls start as
                # soon as (wq chunk 0, xT chunk 0) land.
                wqr = wq.rearrange("(k p) m -> p k m", p=128)
                xr = xT.rearrange("(k p) s -> p k s", p=128)
                for k in range(KC):
                    nc.scalar.dma_start(out=wq_sb[:, k, :], in_=wqr[:, k, :])
                    nc.sync.dma_start(out=xt_sb[:, k, :], in_=xr[:, k, :])
                nc.scalar.dma_start(out=cos_sb, in_=cosT[:])
                nc.scalar.dma_start(out=sin_sb, in_=sinT[:])
                nc.scalar.dma_start(out=wk_sb, in_=wk.rearrange("(k p) m -> p k m", p=128))
                nc.scalar.dma_start(out=wv_sb, in_=wv.rearrange("(k p) m -> p k m", p=128))
                nc.scalar.dma_start(out=iden_sb, in_=iden[:])
                nc.scalar.dma_start(out=mask_sb, in_=masks[:])
                nc.scalar.dma_start(out=ones_sb, in_=onesc[:])
                nc.scalar.dma_start(out=fcw_sb, in_=fcw.rearrange("(h p) n -> p h n", p=128))

                def rope_full(dst, tq):
                    # dst/tq: [128, S] fp16; evens in partitions 0:64, odds 64:128.
                    # cos/sin are duplicated across both halves so every
                    # SBUF*SBUF tensor op has equal input base partitions.
                    pe, po = tq[0:64, :], tq[64:128, :]
                    t1 = p1tmp.tile([64, S], F16, name="t1", tag="t1")
                    t2 = p1tmp.tile([64, S], F16, name="t2", tag="t2")
                    nc.vector.tensor_tensor(t1, pe, cos_sb[0:64, :], op=mybir.AluOpType.mult)
                    nc.vector.tensor_tensor(t2, po, sin_sb[64:128, :], op=mybir.AluOpType.mult)
                    nc.vector.tensor_tensor(dst[0:64, :], t1, t2, op=mybir.AluOpType.subtract)
                    t3 = p1tmp.tile([64, S], F16, name="t3", tag="t1")
                    t4 = p1tmp.tile([64, S], F16, name="t4", tag="t2")
                    nc.vector.tensor_tensor(t3, pe, sin_sb[0:64, :], op=mybir.AluOpType.mult)
                    nc.vector.tensor_tensor(t4, po, cos_sb[64:128, :], op=mybir.AluOpType.mult)
                    nc.vector.tensor_tensor(dst[64:128, :], t3, t4, op=mybir.AluOpType.add)

                # Q^T = wq^T @ xT, kc-outer so PE consumes chunks as they land
                for half in range(2):
                    accs = []
                    for mh in (2 * half, 2 * half + 1):
                        for qc in range(SQC):
                            psq = ps8.tile([128, 512], F32, name="psq", tag="pp")
                            accs.append((mh, qc, psq))
                    for k in range(KC):
                        for mh, qc, psq in accs:
                            nc.tensor.matmul(psq, wq_sb[:, k, mh * 128:(mh + 1) * 128],
                                             xt_sb[:, k, qc * 512:(qc + 1) * 512],
                                             start=(k == 0), stop=(k == KC - 1))
                    tqs = {}
                    for mh in (2 * half, 2 * half + 1):
                        tqs[mh] = p1tmp.tile([128, S], F16, name="tq", tag="tq")
                    for mh, qc, psq in accs:
                        nc.scalar.copy(tqs[mh][:, qc * 512:(qc + 1) * 512], psq)
                    for mh in (2 * half, 2 * half + 1):
                        rope_full(qt_sb[:, mh, :], tqs[mh])

                # K^T = wk^T @ xT
                kaccs = [ps8.tile([128, 512], F32, name="psk", tag="pp")
                         for _ in range(SQC)]
                for k in range(KC):
                    for qc in range(SQC):
                        nc.tensor.matmul(kaccs[qc], wk_sb[:, k, :],
                                         xt_sb[:, k, qc * 512:(qc + 1) * 512],
                                         start=(k == 0), stop=(k == KC - 1))
                tk = p1tmp.tile([128, S], F16, name="tk", tag="tq")
                for qc in range(SQC):
                    nc.scalar.copy(tk[:, qc * 512:(qc + 1) * 512], kaccs[qc])
                rope_full(kt_sb, tk)

                # V^T = wv^T @ xT (N=512), then PE-transpose to V [sk, dk]
                vaccs = [ps8.tile([128, 512], F32, name="psvt", tag="pp")
                         for _ in range(SQC)]
                for k in range(KC):
                    for sc in range(SQC):
                        nc.tensor.matmul(vaccs[sc], wv_sb[:, k, :],
                                         xt_sb[:, k, sc * 512:(sc + 1) * 512],
                                         start=(k == 0), stop=(k == KC - 1))
                for sc in range(SQC):
                    nc.scalar.copy(vt_sb[:, sc * 512:(sc + 1) * 512], vaccs[sc])
                for gq in range(4):
                    psv = ps8.tile([128, 512], F16, name="psv", tag="pp")
                    for vt in range(4):
                        skt = gq * 4 + vt
                        nc.tensor.matmul(psv[:, vt * 128:(vt + 1) * 128],
                                         vt_sb[:, skt * 128:(skt + 1) * 128],
                                         iden_sb, is_transpose=True,
                                         start=True, stop=True)
                    nc.vector.tensor_copy(
                        v_sb[:, gq * 4:(gq + 1) * 4, :].rearrange("p a b -> p (a b)"),
                        psv)  # DVE: ACT busy with rope copies by now

            # ---------------- phase 2+3: attention with fc interleaved ----------------
            # qc-outer: after all 4 heads finish a 512-wide query block, the
            # fc matmuls for those rows run — PE work that hides ACT pacing.
            with tc.tile_pool(name="out_sb", bufs=3) as out_sb:

                def fc_block(sqt):
                    # fc for output rows sqt*128..(sqt+1)*128 (PE-dense work
                    # that lets ACT catch up on its exp backlog)
                    ob = out_sb.tile([128, D], F32, name="ob", tag="ob")
                    for nf in range(4):
                        psf = ps8.tile([128, 512], F32, name="psf", tag="pp")
                        for h2 in range(HG):
                            nc.tensor.matmul(psf,
                                             ctxT[:, h2, sqt * 128:(sqt + 1) * 128],
                                             fcw_sb[:, h2, nf * 512:(nf + 1) * 512],
                                             start=(h2 == 0), stop=(h2 == HG - 1))
                        nc.vector.tensor_copy(ob[:, nf * 512:(nf + 1) * 512], psf)
                    nc.sync.dma_start(out=out[sqt * 128:(sqt + 1) * 128, :], in_=ob)

                for qc in range(SQC):
                    for h in range(HG):
                        nkc = 4 * (qc + 1)  # causal: sk chunks 0..nkc-1
                        psc = ps8.tile([128, 512], F32, name="psc", tag="pp")
                        psd = ps8.tile([1, 512], F32, name="psd", tag="pp")
                        qs = qt_sb[:, h, qc * 512:(qc + 1) * 512]
                        es_tiles = [None] * nkc

                        def scores(kc):
                            t = kc - 4 * qc
                            pss = ps8.tile([128, 512], F32, name="pss", tag="pp")
                            es = es_pool.tile([128, 512], F16, name="es", tag="es")
                            z = 128 * t if t > 0 else 0  # dead columns on diag tiles
                            if z:
                                nc.vector.memset(es[:, 0:z], 0.0)
                            nc.tensor.matmul(pss[:, z:512], kt_sb[:, kc * 128:(kc + 1) * 128],
                                             qs[:, z:512], start=True, stop=True)
                            nc.scalar.activation(es[:, z:512], pss[:, z:512],
                                                 mybir.ActivationFunctionType.Exp,
                                                 scale=SCALE)
                            if t >= 0:
                                nc.vector.tensor_tensor(es[:, z:512], es[:, z:512],
                                                        mask_sb[:, t, z:512],
                                                        op=mybir.AluOpType.mult)
                            es_tiles[kc] = es

                        def accum_pv(kc):
                            nc.tensor.matmul(psc, v_sb[:, kc, :], es_tiles[kc],
                                             start=(kc == 0), stop=(kc == nkc - 1))

                        npairs = nkc // 2

                        # software pipeline: scores one pair ahead; the
                        # ones-matmul lags one more pair so PE never waits on
                        # the DVE pair-add
                        dens = []

                        def accum_den_emit(p):
                            esum = es_pool.tile([128, 512], F16, name="esum", tag="esum")
                            nc.vector.tensor_tensor(esum, es_tiles[2 * p],
                                                    es_tiles[2 * p + 1],
                                                    op=mybir.AluOpType.add)
                            dens.append((p, esum))

                        def den_mm(p, esum):
                            nc.tensor.matmul(psd, ones_sb, esum,
                                             start=(p == 0), stop=(p == npairs - 1))

                        scores(0)
                        scores(1)
                        for p in range(npairs):
                            if p + 1 < npairs:
                                scores(2 * p + 2)
                                scores(2 * p + 3)
                            accum_pv(2 * p)
                            accum_pv(2 * p + 1)
                            accum_den_emit(p)
                            if p >= 1:
                                den_mm(*dens[p - 1])
                        den_mm(*dens[npairs - 1])

                        rec = nrm_pool.tile([1, 512], F32, name="rec", tag="rec")
                        nc.vector.reciprocal_approx_fast(rec, psd)
                        rb = nrm_pool.tile([128, 512], F32, name="rb", tag="rb")
                        nc.gpsimd.partition_broadcast(rb, rec)
                        nc.vector.tensor_tensor(ctxT[:, h, qc * 512:(qc + 1) * 512],
                                                psc, rb, op=mybir.AluOpType.mult)

                        if qc >= 1:
                            fc_block((qc - 1) * 4 + h)

                for sqt in range((SQC - 1) * 4, SQC * 4):
                    fc_block(sqt)

            nrm_pool.release()
            es_pool.release()
            ps8.release()

    nc.compile()
    return nc


def _get_compiled():
    global _COMPILED
    if _COMPILED is None:
        _COMPILED = _build()
    return _COMPILED


def _prep_inputs(x, w_q, w_kv, fc_w, fc_b, freqs_cos, freqs_sin):
    x = np.asarray(x, dtype=np.float32)
    w_q = np.asarray(w_q, dtype=np.float32)
    w_kv = np.asarray(w_kv, dtype=np.float32)
    fc_w = np.asarray(fc_w, dtype=np.float32)
    freqs_cos = np.asarray(freqs_cos, dtype=np.float32)
    freqs_sin = np.asarray(freqs_sin, dtype=np.float32)

    # rope pair permutation: evens then odds within each head's DK block
    perm = np.concatenate([np.arange(0, DK, 2), np.arange(1, DK, 2)])

    cosT = np.ascontiguousarray(freqs_cos.T).astype(np.float16)  # [64, S]
    sinT = np.ascontiguousarray(freqs_sin.T).astype(np.float16)
    cosT = np.concatenate([cosT, cosT], axis=0)  # duplicate across halves
    sinT = np.concatenate([sinT, sinT], axis=0)

    # masks[i, t, j] = 1 if i <= j - 128*t  (diagonal tiles, t = kc - 4*qc)
    i_idx = np.arange(128)[:, None, None]
    t_idx = np.arange(4)[None, :, None]
    j_idx = np.arange(512)[None, None, :]
    masks = (i_idx <= j_idx - 128 * t_idx).astype(np.float16)
    onesc = np.ones((128, 1), dtype=np.float16)
    iden = np.eye(128, dtype=np.float16)

    in_maps = []
    for c in range(8):
        b, g = divmod(c, 4)
        xT = np.ascontiguousarray(x[b].T).astype(np.float16)
        wq_g = w_q[:, g * HG * DK:(g + 1) * HG * DK].reshape(D, HG, DK)[:, :, perm]
        wq_g = np.ascontiguousarray(wq_g.reshape(D, HG * DK)).astype(np.float16)
        wk_g = np.ascontiguousarray(w_kv[:, g * DK:(g + 1) * DK][:, perm]).astype(np.float16)
        wv_g = np.ascontiguousarray(w_kv[:, HKV * DK + g * DK:HKV * DK + (g + 1) * DK]).astype(np.float16)
        fcw_g = np.ascontiguousarray(fc_w[g * HG * DK:(g + 1) * HG * DK, :]).astype(np.float16)
        in_maps.append({
            "xT": xT, "wq": wq_g, "wk": wk_g, "wv": wv_g, "fcw": fcw_g,
            "cosT": cosT, "sinT": sinT, "masks": masks, "onesc": onesc,
            "iden": iden,
        })
    return in_maps


_WARMED = False


def kernel_run(trace=False, warmup=True, **inputs):
    global _WARMED
    nc = _get_compiled()
    in_maps = _prep_inputs(**inputs)
    if warmup and not _WARMED:
        # first post-compile execution on a cold device is ~15% slower
        # (table loads / HAM state); do a throwaway run
        run_bass_kernel_spmd(nc, in_maps, core_ids=list(range(8)), trace=False)
        _WARMED = True
    res = run_bass_kernel_spmd(nc, in_maps, core_ids=list(range(8)), trace=trace)
    fc_b = np.asarray(inputs["fc_b"], dtype=np.float32)
    out = np.zeros((B, S, D), dtype=np.float32)
    for c in range(8):
        b = c // 4
        out[b] += res.results[c]["out"]
    out += fc_b[None, None, :]
    return out, res


def kernel(**inputs):
    out, _ = kernel_run(trace=False, **inputs)
    return out

